# revision 13
# baseline (speedup 1.0000x reference)
"""Multi-head attention (B=2, S=2048, DM=1024, H=16, DH=64, causal) on 8 TRN2 cores.

Sharding: tensor-parallel over heads. Core c owns heads {2c, 2c+1} = q/k/v dims
[128c, 128c+128). Each core computes its QKV projections, causal attention for
its 2 heads (both batches), and a partial output projection (row-parallel over
Wo). Host unshards by summing the 8 partials and adding bo (the TP all-reduce).

In-kernel layout (per core):
  - xT (1024 feat, 4096 seq=b*2048+s) bf16, replicated.
  - QT/KT: (128 qdim [h0 d0-63 | h1 d0-63], 4096 seq) bf16 in SBUF.
  - V: natural layout (128 seq, 128 vdim) tiles in SBUF.
  - scoresT tiles: (128 keys, Q_W queries) PSUM via matmul(lhsT=KT_tile, rhs=QT),
    2 heads packed on row-groups (K=64 each, concurrent).
  - exp on ScalarE, ganged over 3 PSUM banks per instruction.
  - AV: matmul(lhsT=V_tile[:, h*64:...], rhs=expT) col-packed 2 heads ->
    ctxT (128 [h0 d | h1 d], Q_W) accumulated in PSUM.
  - softmax denominators: ones-vector matmuls col-packed at partitions 0/32.
  - normalize: DVE reciprocal + DMA partition-broadcast + DVE multiply.
  - out: matmul(lhsT=ctxT_sb, rhs=WoT_slice) -> (128 q, 1024) fp32 partial.

Causality is hardcoded (the reference's attention_mask is always triu causal);
above-diagonal tiles are skipped entirely, diagonal tiles get a triangular
bf16 multiplicative mask.
"""

import os
import sys

import numpy as np

try:
    import concourse  # noqa: F401
except ImportError:
    sys.path.insert(0, "/opt/trn_rl_repo")

import ml_dtypes

BF16 = ml_dtypes.bfloat16

B, S, DM = 2, 2048, 1024
H, DH = 16, 64
NCORES = 8
CPC = DM // NCORES  # 128 q/k/v dims per core (2 heads)
BS = B * S  # 4096
Q_W = 256  # query-block width
GANG = 2  # k-tiles per exp gang (h0 -> PSUM bank 0, h1 -> bank 1)

_CACHE = {}
LAST_EXEC_NS = None
LAST_RESULTS = None


def _build():
    import concourse.bass as bass
    import concourse.mybir as mybir
    from concourse import bacc
    from concourse import tile
    from concourse.masks import make_upper_triangular

    f32 = mybir.dt.float32
    bf16 = mybir.dt.bfloat16
    Exp = mybir.ActivationFunctionType.Exp
    Ident = mybir.ActivationFunctionType.Identity

    nc = bacc.Bacc(
        "TRN2",
        target_bir_lowering=False,
        debug=False,
        enable_asserts=False,
        num_devices=NCORES,
    )

    xT = nc.dram_tensor("xT", (DM, BS), bf16, kind="ExternalInput").ap()
    wq = nc.dram_tensor("wq", (DM, CPC), bf16, kind="ExternalInput").ap()
    wk = nc.dram_tensor("wk", (DM, CPC), bf16, kind="ExternalInput").ap()
    wv = nc.dram_tensor("wv", (DM, CPC), bf16, kind="ExternalInput").ap()
    wo = nc.dram_tensor("wo", (CPC, DM), bf16, kind="ExternalInput").ap()
    bq = nc.dram_tensor("bq", (CPC, 1), f32, kind="ExternalInput").ap()
    bk = nc.dram_tensor("bk", (CPC, 1), f32, kind="ExternalInput").ap()
    bv = nc.dram_tensor("bv", (1, CPC), bf16, kind="ExternalInput").ap()
    out = nc.dram_tensor("out", (BS, DM), f32, kind="ExternalOutput").ap()

    KT_FEAT = DM // 128  # 8 contraction tiles

    with tile.TileContext(nc) as tc:
        with tc.tile_pool(name="consts", bufs=1) as consts, \
             tc.tile_pool(name="sb", bufs=2) as sb, \
             tc.tile_pool(name="psp", bufs=1, space="PSUM") as psp:

            # ---- persistent tiles -------------------------------------------------
            wq_sb = consts.tile((128, KT_FEAT, CPC), bf16, name="wq_sb")
            wk_sb = consts.tile((128, KT_FEAT, CPC), bf16, name="wk_sb")
            wv_sb = consts.tile((128, KT_FEAT, CPC), bf16, name="wv_sb")
            wo_sb = consts.tile((CPC, DM), bf16, name="wo_sb")
            bq_sb = consts.tile((CPC, 1), f32, name="bq_sb")
            bk_sb = consts.tile((CPC, 1), f32, name="bk_sb")
            bv_sb = consts.tile((1, CPC), bf16, name="bv_sb")
            nc.sync.dma_start(wq_sb[:], wq.rearrange("(t p) m -> p t m", p=128))
            nc.sync.dma_start(wk_sb[:], wk.rearrange("(t p) m -> p t m", p=128))
            nc.sync.dma_start(wv_sb[:], wv.rearrange("(t p) m -> p t m", p=128))
            nc.sync.dma_start(wo_sb[:], wo)
            nc.sync.dma_start(bq_sb[:], bq)
            nc.sync.dma_start(bk_sb[:], bk)
            nc.sync.dma_start(bv_sb[:], bv)

            ones_col = consts.tile((128, 1), bf16, name="ones_col")
            nc.vector.memset(ones_col[:], 1.0)
            ones_row = consts.tile((1, 128), bf16, name="ones_row")
            nc.vector.memset(ones_row[:], 1.0)
            ones_f33 = consts.tile((33, 128), f32, name="ones_f33")
            nc.vector.memset(ones_f33[:], 1.0)
            # causal mask tile: mask[k, q] = 1.0 if k <= q else 0.0
            cmask = consts.tile((128, 128), bf16, name="cmask")
            make_upper_triangular(nc, cmask[:], val=1.0, diag=True)

            QT_sb = consts.tile((128, BS), bf16, name="QT_sb")
            KT_sb = consts.tile((128, BS), bf16, name="KT_sb")
            V_sb = consts.tile((128, BS // 128, 128), bf16, name="V_sb")

            xT3 = xT.rearrange("(t p) q -> p t q", p=128)

            # ---- QKV projections --------------------------------------------------
            N_CH = BS // 512  # 8 chunks of 512 sequence positions
            for ch in range(N_CH):
                c0 = ch * 512
                xchunk = sb.tile((128, KT_FEAT, 512), bf16, name=f"xchunk{ch}",
                                 tag="xchunk", bufs=2)
                nc.sync.dma_start(xchunk[:], xT3[:, :, c0:c0 + 512])

                for pname, w_sb, b_sb, out_T in (
                    ("q", wq_sb, bq_sb, QT_sb),
                    ("k", wk_sb, bk_sb, KT_sb),
                ):
                    ps_p = psp.tile((128, 1024), f32, name=f"ps_{pname}{ch}",
                                    tag="big", bufs=3)
                    for t in range(KT_FEAT):
                        nc.tensor.matmul(
                            ps_p[:, :512],
                            lhsT=w_sb[:, t, :],
                            rhs=xchunk[:, t, :],
                            start=(t == 0),
                            stop=(t == KT_FEAT - 1),
                        )
                    nc.scalar.activation(
                        out_T[:, c0:c0 + 512], ps_p[:, :512], Ident, bias=b_sb[:],
                    )

                # V in natural layout: 4 seq-subtiles of 128
                # NOTE: start=True lazily zeroes the whole 2KB PSUM zero region
                # (bank row) for the written partitions, so only the very first
                # matmul touching this bank may set it.
                ps_v = psp.tile((128, 1024), f32, name=f"ps_v{ch}",
                                tag="big", bufs=3)
                for sub in range(4):
                    lo = sub * 128
                    for t in range(KT_FEAT):
                        nc.tensor.matmul(
                            ps_v[:, lo:lo + 128],
                            lhsT=xchunk[:, t, lo:lo + 128],
                            rhs=wv_sb[:, t, :],
                            start=(t == 0 and sub == 0),
                            stop=False,
                            skip_group_check=True,
                        )
                    # += 1 x bv  (bias via rank-1 matmul)
                    nc.tensor.matmul(
                        ps_v[:, lo:lo + 128],
                        lhsT=ones_row[:1, :],
                        rhs=bv_sb[:1, :],
                        start=False,
                        stop=(sub == 3),
                        skip_group_check=True,
                    )
                nc.vector.tensor_copy(
                    V_sb[:, ch * 4:(ch + 1) * 4, :],
                    ps_v[:, :512].rearrange("p (s v) -> p s v", v=128),
                )

            # ---- attention + output projection ------------------------------------
            NQB = S // Q_W  # q-blocks per batch
            for b in range(B):
                for qb in range(NQB):
                    qb0 = qb * Q_W
                    g0 = b * S + qb0  # global query start
                    n_t = (qb0 + Q_W) // 128  # causal: k-tiles needed

                    ps_ctx = psp.tile((128, Q_W), f32, name=f"ps_ctx{b}_{qb}",
                                      tag="ctx", bufs=1)
                    ps_sums = psp.tile((128, Q_W), f32, name=f"ps_sums{b}_{qb}",
                                       tag="sums", bufs=1)

                    for gs in range(0, n_t, GANG):
                        gts = list(range(gs, min(gs + GANG, n_t)))
                        # Score slots packed per head: h0 in PSUM bank 0
                        # ([0:512)), h1 in bank 1 ([512:1024)). The two heads'
                        # score matmuls run CONCURRENTLY on different PE
                        # row-groups, and concurrent matmuls faulting into the
                        # same PSUM bank crash the core -- so the heads must
                        # land in different banks. Diag-clipped tiles pack
                        # tightly so the ganged exp reads only written PSUM.
                        slots = []
                        htot = 0
                        for h in range(2):
                            base = h * 512
                            for t in gts:
                                off = max(0, 128 * t - qb0)  # causal left-clip
                                slots.append((t, h, base, off, Q_W - off))
                                base += Q_W - off
                            htot = base - h * 512
                        ps_s = psp.tile((128, 1024), f32, name=f"ps_s{b}_{qb}_{gs}",
                                        tag="big", bufs=3)
                        exp_sb = sb.tile((128, 1024), bf16, name=f"exp{b}_{qb}_{gs}",
                                         tag="exp", bufs=3)
                        for t, h, sb0, off, w in slots:
                            k0 = 128 * t
                            # start only on the first slot of each PSUM bank
                            # (slots never straddle banks; see packing above)
                            nc.tensor.matmul(
                                ps_s[:, sb0:sb0 + w],
                                lhsT=KT_sb[h * 64:(h + 1) * 64,
                                           b * S + k0:b * S + k0 + 128],
                                rhs=QT_sb[h * 64:(h + 1) * 64,
                                          g0 + off:g0 + Q_W],
                                start=(sb0 % 512 == 0),
                                stop=True,
                                tile_position=(h * 64, 0),
                                skip_group_check=True,
                            )
                        if htot == 512:
                            nc.scalar.activation(
                                exp_sb[:, :1024], ps_s[:, :1024], Exp, scale=0.125,
                            )
                        else:
                            nc.scalar.activation(
                                exp_sb[:, :htot], ps_s[:, :htot], Exp, scale=0.125,
                            )
                            nc.scalar.activation(
                                exp_sb[:, 512:512 + htot], ps_s[:, 512:512 + htot],
                                Exp, scale=0.125,
                            )
                        for t, h, sb0, off, w in slots:
                            k0 = 128 * t
                            first = t == 0
                            last = t == n_t - 1
                            if k0 >= qb0:  # diagonal tile: triangular mask
                                nc.vector.tensor_mul(
                                    exp_sb[:, sb0:sb0 + 128],
                                    exp_sb[:, sb0:sb0 + 128],
                                    cmask[:],
                                )
                            nc.tensor.matmul(
                                ps_ctx[h * 64:(h + 1) * 64, off:Q_W],
                                lhsT=V_sb[:, (b * S + k0) // 128,
                                          h * 64:(h + 1) * 64],
                                rhs=exp_sb[:, sb0:sb0 + w],
                                start=first,
                                stop=last,
                                tile_position=(0, h * 64),
                                skip_group_check=True,
                            )
                            nc.tensor.matmul(
                                ps_sums[h * 32:h * 32 + 1, off:Q_W],
                                lhsT=ones_col[:, :1],
                                rhs=exp_sb[:, sb0:sb0 + w],
                                start=first,
                                stop=last,
                                tile_position=(0, h * 32),
                                skip_group_check=True,
                            )

                    # normalize: ctx_sb = ps_ctx * (1/sums) broadcast over partitions
                    rec = sb.tile((33, Q_W), f32, name=f"rec{b}_{qb}",
                                  tag="rec", bufs=2)
                    nc.vector.reciprocal(rec[0:1, :], ps_sums[0:1, :])
                    nc.vector.reciprocal(rec[32:33, :], ps_sums[32:33, :])
                    # broadcast the two reciprocal rows across partitions via
                    # K=1 ones-matmuls (col-packed); reuses the sums PSUM slot
                    ps_bc = psp.tile((128, Q_W), f32, name=f"ps_bc{b}_{qb}",
                                     tag="sums", bufs=1)
                    nc.tensor.matmul(
                        ps_bc[0:64, :], lhsT=ones_f33[0:1, 0:64], rhs=rec[0:1, :],
                        start=True, stop=True, tile_position=(0, 0),
                        skip_group_check=True,
                    )
                    nc.tensor.matmul(
                        ps_bc[64:128, :], lhsT=ones_f33[32:33, 64:128],
                        rhs=rec[32:33, :],
                        start=True, stop=True, tile_position=(32, 64),
                        skip_group_check=True,
                    )
                    rec_bc = sb.tile((128, Q_W), f32, name=f"recbc{b}_{qb}",
                                     tag="recbc", bufs=2)
                    nc.scalar.copy(rec_bc[:], ps_bc[:])
                    ctx_sb = sb.tile((128, Q_W), bf16, name=f"ctx{b}_{qb}",
                                     tag="ctx_sb", bufs=2)
                    nc.vector.tensor_mul(ctx_sb[:], ps_ctx[:], rec_bc[:])

                    # output projection: partial_out = ctxT.T @ woT
                    for sub in range(Q_W // 128):
                        ps_o = psp.tile((128, 1024), f32, name=f"ps_o{b}_{qb}_{sub}",
                                        tag="big", bufs=3)
                        for nn in range(2):
                            nc.tensor.matmul(
                                ps_o[:, nn * 512:(nn + 1) * 512],
                                lhsT=ctx_sb[:, sub * 128:(sub + 1) * 128],
                                rhs=wo_sb[:, nn * 512:(nn + 1) * 512],
                                start=True,
                                stop=True,
                            )
                        o_sb = sb.tile((128, DM), f32, name=f"o_sb{b}_{qb}_{sub}",
                                       tag="o_sb", bufs=3)
                        nc.vector.tensor_copy(o_sb[:], ps_o[:, :DM])
                        r0 = g0 + sub * 128
                        nc.sync.dma_start(out[r0:r0 + 128, :], o_sb[:])

    nc.compile()
    return nc


def _prep_inputs(x, Wq, bq, Wk, bk, Wv, bv, Wo):
    """Build the 8 per-core input maps (host-side sharding)."""
    x = np.asarray(x, dtype=np.float32)
    xT = np.ascontiguousarray(x.reshape(BS, DM).T).astype(BF16)
    in_maps = []
    for c in range(NCORES):
        sl = slice(c * CPC, (c + 1) * CPC)
        in_maps.append({
            "xT": xT,
            "wq": np.ascontiguousarray(np.asarray(Wq, np.float32)[sl, :].T).astype(BF16),
            "wk": np.ascontiguousarray(np.asarray(Wk, np.float32)[sl, :].T).astype(BF16),
            "wv": np.ascontiguousarray(np.asarray(Wv, np.float32)[sl, :].T).astype(BF16),
            "wo": np.ascontiguousarray(np.asarray(Wo, np.float32)[:, sl].T).astype(BF16),
            "bq": np.asarray(bq, np.float32)[sl].reshape(CPC, 1).copy(),
            "bk": np.asarray(bk, np.float32)[sl].reshape(CPC, 1).copy(),
            "bv": np.asarray(bv, np.float32)[sl].reshape(1, CPC).astype(BF16),
        })
    return in_maps


def _run(in_maps, trace=False):
    global LAST_EXEC_NS, LAST_RESULTS
    from concourse import bass_utils

    if "nc" not in _CACHE:
        _CACHE["nc"] = _build()
    nc = _CACHE["nc"]
    res = bass_utils.run_bass_kernel_spmd(
        nc, in_maps, core_ids=list(range(NCORES)), trace=trace,
    )
    LAST_EXEC_NS = getattr(res, "exec_time_ns", None)
    LAST_RESULTS = res
    return res.results


def kernel(x, Wq, bq, Wk, bk, Wv, bv, Wo, bo, attention_mask=None, _trace=False):
    """Full inputs in, full output out. attention_mask is the reference's causal
    mask; causality is hardcoded in the kernel."""
    in_maps = _prep_inputs(x, Wq, bq, Wk, bk, Wv, bv, Wo)
    results = _run(in_maps, trace=_trace)
    acc = np.zeros((BS, DM), dtype=np.float32)
    for c in range(NCORES):
        acc += results[c]["out"]
    acc += np.asarray(bo, np.float32)[None, :]
    return acc.reshape(B, S, DM)


# revision 16
# speedup vs baseline: 2.0808x; 2.0808x over previous
"""Multi-head attention (B=2, S=2048, DM=1024, H=16, DH=64, causal) on 8 TRN2 cores.

Sharding: tensor-parallel over heads. Core c owns heads {2c, 2c+1} = q/k/v dims
[128c, 128c+128). Each core computes its QKV projections, causal attention for
its 2 heads (both batches), and a partial output projection (row-parallel over
Wo). Host unshards by summing the 8 partials and adding bo (the TP all-reduce).

In-kernel layout (per core):
  - xT (1024 feat, 4096 seq=b*2048+s) bf16, replicated.
  - QT/KT: (128 qdim [h0 d0-63 | h1 d0-63], 4096 seq) bf16 in SBUF.
  - V: natural layout (128 seq, 128 vdim) tiles in SBUF.
  - scoresT tiles: (128 keys, Q_W queries) PSUM via matmul(lhsT=KT_tile, rhs=QT),
    2 heads packed on row-groups (K=64 each, concurrent).
  - exp on ScalarE, ganged over 3 PSUM banks per instruction.
  - AV: matmul(lhsT=V_tile[:, h*64:...], rhs=expT) col-packed 2 heads ->
    ctxT (128 [h0 d | h1 d], Q_W) accumulated in PSUM.
  - softmax denominators: ones-vector matmuls col-packed at partitions 0/32.
  - normalize: DVE reciprocal + DMA partition-broadcast + DVE multiply.
  - out: matmul(lhsT=ctxT_sb, rhs=WoT_slice) -> (128 q, 1024) fp32 partial.

Causality is hardcoded (the reference's attention_mask is always triu causal);
above-diagonal tiles are skipped entirely, diagonal tiles get a triangular
bf16 multiplicative mask.
"""

import os
import sys

import numpy as np

try:
    import concourse  # noqa: F401
except ImportError:
    sys.path.insert(0, "/opt/trn_rl_repo")

import ml_dtypes

BF16 = ml_dtypes.bfloat16

B, S, DM = 2, 2048, 1024
H, DH = 16, 64
NCORES = 8
CPC = DM // NCORES  # 128 q/k/v dims per core (2 heads)
BS = B * S  # 4096
Q_W = 256  # query-block width
GANG = 2  # k-tiles per exp gang (h0 -> PSUM bank 0, h1 -> bank 1)

_CACHE = {}
LAST_EXEC_NS = None
LAST_RESULTS = None


def _build():
    import concourse.bass as bass
    import concourse.mybir as mybir
    from concourse import bacc
    from concourse import tile
    from concourse.masks import make_upper_triangular

    f32 = mybir.dt.float32
    bf16 = mybir.dt.bfloat16
    Exp = mybir.ActivationFunctionType.Exp
    Ident = mybir.ActivationFunctionType.Identity

    nc = bacc.Bacc(
        "TRN2",
        target_bir_lowering=False,
        debug=False,
        enable_asserts=False,
        num_devices=NCORES,
    )

    xT = nc.dram_tensor("xT", (DM, BS), bf16, kind="ExternalInput").ap()
    wq = nc.dram_tensor("wq", (DM, CPC), bf16, kind="ExternalInput").ap()
    wk = nc.dram_tensor("wk", (DM, CPC), bf16, kind="ExternalInput").ap()
    wv = nc.dram_tensor("wv", (DM, CPC), bf16, kind="ExternalInput").ap()
    wo = nc.dram_tensor("wo", (CPC, DM), bf16, kind="ExternalInput").ap()
    bq = nc.dram_tensor("bq", (CPC, 1), f32, kind="ExternalInput").ap()
    bk = nc.dram_tensor("bk", (CPC, 1), f32, kind="ExternalInput").ap()
    bv = nc.dram_tensor("bv", (1, CPC), bf16, kind="ExternalInput").ap()
    out = nc.dram_tensor("out", (BS, DM), f32, kind="ExternalOutput").ap()

    KT_FEAT = DM // 128  # 8 contraction tiles

    with tile.TileContext(nc) as tc:
        with tc.tile_pool(name="consts", bufs=1) as consts, \
             tc.tile_pool(name="sb", bufs=2) as sb, \
             tc.tile_pool(name="psp", bufs=1, space="PSUM") as psp:

            # ---- persistent tiles -------------------------------------------------
            wq_sb = consts.tile((128, KT_FEAT, CPC), bf16, name="wq_sb")
            wk_sb = consts.tile((128, KT_FEAT, CPC), bf16, name="wk_sb")
            wv_sb = consts.tile((128, KT_FEAT, CPC), bf16, name="wv_sb")
            wo_sb = consts.tile((CPC, DM), bf16, name="wo_sb")
            bq_sb = consts.tile((CPC, 1), f32, name="bq_sb")
            bk_sb = consts.tile((CPC, 1), f32, name="bk_sb")
            bv_sb = consts.tile((1, CPC), bf16, name="bv_sb")
            nc.sync.dma_start(wq_sb[:], wq.rearrange("(t p) m -> p t m", p=128))
            nc.sync.dma_start(wk_sb[:], wk.rearrange("(t p) m -> p t m", p=128))
            nc.sync.dma_start(wv_sb[:], wv.rearrange("(t p) m -> p t m", p=128))
            nc.sync.dma_start(wo_sb[:], wo)
            nc.sync.dma_start(bq_sb[:], bq)
            nc.sync.dma_start(bk_sb[:], bk)
            nc.sync.dma_start(bv_sb[:], bv)

            ones_col = consts.tile((128, 1), bf16, name="ones_col")
            nc.vector.memset(ones_col[:], 1.0)
            ones_row = consts.tile((1, 128), bf16, name="ones_row")
            nc.vector.memset(ones_row[:], 1.0)
            ones_f33 = consts.tile((33, 128), f32, name="ones_f33")
            nc.vector.memset(ones_f33[:], 1.0)
            # causal mask tile: mask[k, q] = 1.0 if k <= q else 0.0
            cmask = consts.tile((128, 128), bf16, name="cmask")
            make_upper_triangular(nc, cmask[:], val=1.0, diag=True)

            QT_sb = consts.tile((128, BS), bf16, name="QT_sb")
            KT_sb = consts.tile((128, BS), bf16, name="KT_sb")
            V_sb = consts.tile((128, BS // 128, 128), bf16, name="V_sb")

            xT3 = xT.rearrange("(t p) q -> p t q", p=128)

            # ---- QKV projections --------------------------------------------------
            N_CH = BS // 512  # 8 chunks of 512 sequence positions
            for ch in range(N_CH):
                c0 = ch * 512
                xchunk = sb.tile((128, KT_FEAT, 512), bf16, name=f"xchunk{ch}",
                                 tag="xchunk", bufs=2)
                nc.sync.dma_start(xchunk[:], xT3[:, :, c0:c0 + 512])

                for pname, w_sb, b_sb, out_T in (
                    ("q", wq_sb, bq_sb, QT_sb),
                    ("k", wk_sb, bk_sb, KT_sb),
                ):
                    ps_p = psp.tile((128, 1024), f32, name=f"ps_{pname}{ch}",
                                    tag="big", bufs=3)
                    for t in range(KT_FEAT):
                        nc.tensor.matmul(
                            ps_p[:, :512],
                            lhsT=w_sb[:, t, :],
                            rhs=xchunk[:, t, :],
                            start=(t == 0),
                            stop=(t == KT_FEAT - 1),
                        )
                    nc.scalar.activation(
                        out_T[:, c0:c0 + 512], ps_p[:, :512], Ident, bias=b_sb[:],
                    )

                # V in natural layout: 4 seq-subtiles of 128
                # NOTE: start=True lazily zeroes the whole 2KB PSUM zero region
                # (bank row) for the written partitions, so only the very first
                # matmul touching this bank may set it.
                ps_v = psp.tile((128, 1024), f32, name=f"ps_v{ch}",
                                tag="big", bufs=3)
                for sub in range(4):
                    lo = sub * 128
                    for t in range(KT_FEAT):
                        nc.tensor.matmul(
                            ps_v[:, lo:lo + 128],
                            lhsT=xchunk[:, t, lo:lo + 128],
                            rhs=wv_sb[:, t, :],
                            start=(t == 0 and sub == 0),
                            stop=False,
                            skip_group_check=True,
                        )
                    # += 1 x bv  (bias via rank-1 matmul)
                    nc.tensor.matmul(
                        ps_v[:, lo:lo + 128],
                        lhsT=ones_row[:1, :],
                        rhs=bv_sb[:1, :],
                        start=False,
                        stop=(sub == 3),
                        skip_group_check=True,
                    )
                nc.vector.tensor_copy(
                    V_sb[:, ch * 4:(ch + 1) * 4, :],
                    ps_v[:, :512].rearrange("p (s v) -> p s v", v=128),
                )

            # ---- attention + output projection ------------------------------------
            NQB = S // Q_W  # q-blocks per batch
            for b in range(B):
                for qb in range(NQB):
                    qb0 = qb * Q_W
                    g0 = b * S + qb0  # global query start
                    n_t = (qb0 + Q_W) // 128  # causal: k-tiles needed

                    ps_ctx = psp.tile((128, Q_W), f32, name=f"ps_ctx{b}_{qb}",
                                      tag="ctx", bufs=1)
                    ps_sums = psp.tile((128, Q_W), f32, name=f"ps_sums{b}_{qb}",
                                       tag="sums", bufs=1)

                    for gs in range(0, n_t, GANG):
                        gts = list(range(gs, min(gs + GANG, n_t)))
                        # Score slots packed per head: h0 in PSUM bank 0
                        # ([0:512)), h1 in bank 1 ([512:1024)). The two heads'
                        # score matmuls run CONCURRENTLY on different PE
                        # row-groups, and concurrent matmuls faulting into the
                        # same PSUM bank crash the core -- so the heads must
                        # land in different banks. Diag-clipped tiles pack
                        # tightly so the ganged exp reads only written PSUM.
                        slots = []
                        htot = 0
                        for h in range(2):
                            base = h * 512
                            for t in gts:
                                off = max(0, 128 * t - qb0)  # causal left-clip
                                slots.append((t, h, base, off, Q_W - off))
                                base += Q_W - off
                            htot = base - h * 512
                        ps_s = psp.tile((128, 1024), f32, name=f"ps_s{b}_{qb}_{gs}",
                                        tag="big", bufs=3)
                        exp_sb = sb.tile((128, 1024), bf16, name=f"exp{b}_{qb}_{gs}",
                                         tag="exp", bufs=3)
                        for t, h, sb0, off, w in slots:
                            k0 = 128 * t
                            # start only on the first slot of each PSUM bank
                            # (slots never straddle banks; see packing above)
                            nc.tensor.matmul(
                                ps_s[:, sb0:sb0 + w],
                                lhsT=KT_sb[h * 64:(h + 1) * 64,
                                           b * S + k0:b * S + k0 + 128],
                                rhs=QT_sb[h * 64:(h + 1) * 64,
                                          g0 + off:g0 + Q_W],
                                start=(sb0 % 512 == 0),
                                stop=True,
                                tile_position=(h * 64, 0),
                                skip_group_check=True,
                            )
                        if htot == 512:
                            nc.scalar.activation(
                                exp_sb[:, :1024], ps_s[:, :1024], Exp, scale=0.125,
                            )
                        else:
                            nc.scalar.activation(
                                exp_sb[:, :htot], ps_s[:, :htot], Exp, scale=0.125,
                            )
                            nc.scalar.activation(
                                exp_sb[:, 512:512 + htot], ps_s[:, 512:512 + htot],
                                Exp, scale=0.125,
                            )
                        for t, h, sb0, off, w in slots:
                            k0 = 128 * t
                            first = t == 0
                            last = t == n_t - 1
                            if k0 >= qb0:  # diagonal tile: triangular mask
                                nc.vector.tensor_mul(
                                    exp_sb[:, sb0:sb0 + 128],
                                    exp_sb[:, sb0:sb0 + 128],
                                    cmask[:],
                                )
                            nc.tensor.matmul(
                                ps_ctx[h * 64:(h + 1) * 64, off:Q_W],
                                lhsT=V_sb[:, (b * S + k0) // 128,
                                          h * 64:(h + 1) * 64],
                                rhs=exp_sb[:, sb0:sb0 + w],
                                start=first,
                                stop=last,
                                tile_position=(0, h * 64),
                                skip_group_check=True,
                            )
                            nc.tensor.matmul(
                                ps_sums[h * 32:h * 32 + 1, off:Q_W],
                                lhsT=ones_col[:, :1],
                                rhs=exp_sb[:, sb0:sb0 + w],
                                start=first,
                                stop=last,
                                tile_position=(0, h * 32),
                                skip_group_check=True,
                            )

                    # normalize: ctx_sb = ps_ctx * (1/sums) broadcast over partitions
                    rec = sb.tile((33, Q_W), f32, name=f"rec{b}_{qb}",
                                  tag="rec", bufs=2)
                    nc.vector.reciprocal(rec[0:1, :], ps_sums[0:1, :])
                    nc.vector.reciprocal(rec[32:33, :], ps_sums[32:33, :])
                    # broadcast the two reciprocal rows across partitions via
                    # K=1 ones-matmuls (col-packed); reuses the sums PSUM slot
                    ps_bc = psp.tile((128, Q_W), f32, name=f"ps_bc{b}_{qb}",
                                     tag="sums", bufs=1)
                    nc.tensor.matmul(
                        ps_bc[0:64, :], lhsT=ones_f33[0:1, 0:64], rhs=rec[0:1, :],
                        start=True, stop=True, tile_position=(0, 0),
                        skip_group_check=True,
                    )
                    nc.tensor.matmul(
                        ps_bc[64:128, :], lhsT=ones_f33[32:33, 64:128],
                        rhs=rec[32:33, :],
                        start=True, stop=True, tile_position=(32, 64),
                        skip_group_check=True,
                    )
                    rec_bc = sb.tile((128, Q_W), f32, name=f"recbc{b}_{qb}",
                                     tag="recbc", bufs=2)
                    nc.vector.tensor_copy(rec_bc[:], ps_bc[:])
                    ctx_sb = sb.tile((128, Q_W), bf16, name=f"ctx{b}_{qb}",
                                     tag="ctx_sb", bufs=2)
                    nc.vector.tensor_mul(ctx_sb[:], ps_ctx[:], rec_bc[:])

                    # output projection: partial_out = ctxT.T @ woT
                    for sub in range(Q_W // 128):
                        ps_o = psp.tile((128, 1024), f32, name=f"ps_o{b}_{qb}_{sub}",
                                        tag="big", bufs=3)
                        for nn in range(2):
                            nc.tensor.matmul(
                                ps_o[:, nn * 512:(nn + 1) * 512],
                                lhsT=ctx_sb[:, sub * 128:(sub + 1) * 128],
                                rhs=wo_sb[:, nn * 512:(nn + 1) * 512],
                                start=True,
                                stop=True,
                            )
                        o_sb = sb.tile((128, DM), f32, name=f"o_sb{b}_{qb}_{sub}",
                                       tag="o_sb", bufs=3)
                        nc.vector.tensor_copy(o_sb[:], ps_o[:, :DM])
                        r0 = g0 + sub * 128
                        nc.sync.dma_start(out[r0:r0 + 128, :], o_sb[:])

    nc.compile()
    return nc


def _prep_inputs(x, Wq, bq, Wk, bk, Wv, bv, Wo):
    """Build the 8 per-core input maps (host-side sharding)."""
    x = np.asarray(x, dtype=np.float32)
    xT = np.ascontiguousarray(x.reshape(BS, DM).T).astype(BF16)
    in_maps = []
    for c in range(NCORES):
        sl = slice(c * CPC, (c + 1) * CPC)
        in_maps.append({
            "xT": xT,
            "wq": np.ascontiguousarray(np.asarray(Wq, np.float32)[sl, :].T).astype(BF16),
            "wk": np.ascontiguousarray(np.asarray(Wk, np.float32)[sl, :].T).astype(BF16),
            "wv": np.ascontiguousarray(np.asarray(Wv, np.float32)[sl, :].T).astype(BF16),
            "wo": np.ascontiguousarray(np.asarray(Wo, np.float32)[:, sl].T).astype(BF16),
            "bq": np.asarray(bq, np.float32)[sl].reshape(CPC, 1).copy(),
            "bk": np.asarray(bk, np.float32)[sl].reshape(CPC, 1).copy(),
            "bv": np.asarray(bv, np.float32)[sl].reshape(1, CPC).astype(BF16),
        })
    return in_maps


def _run(in_maps, trace=False):
    global LAST_EXEC_NS, LAST_RESULTS
    from concourse import bass_utils

    if "nc" not in _CACHE:
        _CACHE["nc"] = _build()
    nc = _CACHE["nc"]
    res = bass_utils.run_bass_kernel_spmd(
        nc, in_maps, core_ids=list(range(NCORES)), trace=trace,
    )
    LAST_EXEC_NS = getattr(res, "exec_time_ns", None)
    LAST_RESULTS = res
    return res.results


def kernel(x, Wq, bq, Wk, bk, Wv, bv, Wo, bo, attention_mask=None, _trace=False):
    """Full inputs in, full output out. attention_mask is the reference's causal
    mask; causality is hardcoded in the kernel."""
    in_maps = _prep_inputs(x, Wq, bq, Wk, bk, Wv, bv, Wo)
    results = _run(in_maps, trace=_trace)
    acc = np.zeros((BS, DM), dtype=np.float32)
    for c in range(NCORES):
        acc += results[c]["out"]
    acc += np.asarray(bo, np.float32)[None, :]
    return acc.reshape(B, S, DM)
